# revision 1
# baseline (speedup 1.0000x reference)
"""GCN message-passing kernel for 8 Trainium2 NeuronCores.

Strategy (graph/data parallel, per the sharding hint):
  - Destination nodes are sharded across the 8 cores in contiguous ranges.
  - Within each core, its destinations are dealt (by in-degree, snake order)
    into 128-wide blocks so per-block edge counts are balanced across
    blocks AND cores (the SPMD program has compile-time-fixed loop bounds).
  - Per layer: each core computes hs = dinv * (x W^T + b) for its own node
    shard (PE transpose + matmul), downcasts to bf16, and the shards are
    AllGathered into a full [C*NPAD, 128] bf16 table in DRAM.
  - Messages are fetched with batched indirect DMA gathers (one SWDGE
    instruction per ~hundred 128-edge tiles) and scatter-added per
    destination block with a one-hot matmul:
        agg_block[d, f] += S_tile[e, d]^T @ msg_tile[e, f]
    accumulated in PSUM. S_tile is built on the DVE with a single
    is_equal tensor_scalar of an iota row against per-edge dest ranks.
  - BN statistics (sum, sum of squares) are computed with mask-vector
    matmuls over the aggregated blocks and AllReduced across cores; the
    apply (scale/shift + relu + residual) runs on full-shard DVE/ACT ops.

kernel(**inputs) takes the FULL inputs and returns the FULL output.
"""

import numpy as np
import ml_dtypes

import concourse.bacc as bacc
import concourse.bass as bass
import concourse.mybir as mybir
import concourse.tile as tile
from concourse.bass_utils import run_bass_kernel_spmd
from concourse.masks import make_identity

P = 128
F32 = mybir.dt.float32
BF16 = mybir.dt.bfloat16
AF = mybir.ActivationFunctionType
ALU = mybir.AluOpType


class Cfg:
    def __init__(self, N, E, D, L, C, bpc, kg=4, bn_eps=1e-5):
        assert D == 128
        self.N, self.E, self.D, self.L, self.C = N, E, D, L, C
        self.NSH = N // C                      # real nodes per core
        assert self.NSH * C == N
        self.TPC = (self.NSH + P - 1) // P     # node tiles (blocks) per core
        self.NPAD = self.TPC * P               # padded nodes per core
        assert self.NSH < self.NPAD, "need at least one guaranteed-zero pad row"
        self.TROWS = C * self.NPAD             # gather table rows
        self.BPC = bpc                         # blocks per gather chunk
        self.chunks = [
            list(range(i, min(i + bpc, self.TPC))) for i in range(0, self.TPC, bpc)
        ]
        self.BN_EPS = bn_eps
        self.KG = kg  # max idxs per dma_gather call (in 128-edge tiles)
        self.ZROW = self.NSH  # core 0's first pad row: always written as zero
        self.LO = 32768
        if self.TROWS > self.LO:
            c_hi = -((self.LO - self.NSH) // -self.NPAD)
            zhi = c_hi * self.NPAD + self.NSH
            assert self.LO <= zhi < self.TROWS
            self.ZHI = zhi - self.LO
        else:
            self.ZHI = 0


def _preprocess(cfg, x, edge_index, W, b, gamma, beta):
    """All index/layout work on the host. Returns per-core in_maps and the
    (identical across cores) compile-time tile structure."""
    N, C, NSH, NPAD, TPC = cfg.N, cfg.C, cfg.NSH, cfg.NPAD, cfg.TPC
    row = np.asarray(edge_index[0], dtype=np.int64)
    col = np.asarray(edge_index[1], dtype=np.int64)
    x = np.asarray(x, dtype=np.float32)
    deg = np.bincount(row, minlength=N).astype(np.float32)  # out-degree
    deg_in = np.bincount(col, minlength=N)

    # Per-core local permutation: snake-deal destinations (sorted by
    # in-degree desc) into TPC blocks -> balanced per-block edge counts.
    newlocal = np.empty(N, np.int64)
    nblk0 = None
    for c in range(C):
        ids = np.arange(c * NSH, (c + 1) * NSH)
        order = ids[np.argsort(-deg_in[ids], kind="stable")]
        i = np.arange(NSH)
        r, j = i // TPC, i % TPC
        blk = np.where(r % 2 == 1, TPC - 1 - j, j)
        rank = np.zeros(NSH, np.int64)
        cnt = np.zeros(TPC, np.int64)
        for k in range(NSH):
            rank[k] = cnt[blk[k]]
            cnt[blk[k]] += 1
        newlocal[order] = blk * P + rank
        if nblk0 is None:
            nblk0 = cnt.copy()
        else:
            assert (cnt == nblk0).all()
    assert nblk0.max() <= P

    maskv = (np.arange(P)[:, None] < nblk0[None, :]).astype(np.float32)
    table_row = (np.arange(N) // NSH) * NPAD + newlocal  # node -> table row

    e_core = col // NSH
    e_blk = newlocal[col] // P
    e_rank = newlocal[col] % P
    e_src = table_row[row]

    # common tile structure: TLs/THs tiles per block, max over cores/blocks
    split_hi = cfg.TROWS > cfg.LO
    per = {}
    TLs, THs = 1, (1 if split_hi else 0)
    for c in range(C):
        selc = e_core == c
        for lo in (True, False):
            if not lo and not split_hi:
                continue
            sel = selc & ((e_src < cfg.LO) == lo)
            srcs, blks, ranks = e_src[sel], e_blk[sel], e_rank[sel]
            o = np.argsort(blks, kind="stable")
            srcs, blks, ranks = srcs[o], blks[o], ranks[o]
            starts = np.searchsorted(blks, np.arange(TPC))
            ends = np.searchsorted(blks, np.arange(TPC) + 1)
            per[(c, lo)] = (srcs, ranks, starts, ends)
            m = int((-((ends - starts) // -P)).max())
            if lo:
                TLs = max(TLs, m)
            else:
                THs = max(THs, m)
    if not split_hi:
        per = {(c, True): per[(c, True)] for c in range(C)}
    TS = TLs + THs
    NT = TPC * TS
    in_maps = []
    Wt = np.ascontiguousarray(np.transpose(np.asarray(W, np.float32), (0, 2, 1)))
    bT = np.ascontiguousarray(np.asarray(b, np.float32).T)

    def _wrap16(idx):
        w = idx.reshape(-1, 16).T.astype(np.int16)
        return np.ascontiguousarray(np.tile(w, (8, 1)))

    for c in range(C):
        idx_lo = np.full(TPC * TLs * P, cfg.ZROW, np.int64)
        idx_hi = np.full(max(TPC * THs * P, 16), cfg.ZHI, np.int64)
        # one-hot S matrices, block-contiguous: smat[e, (b*TS + t)*P + d]
        smat = np.zeros((P, NT * P), ml_dtypes.bfloat16)
        lo_off = hi_off = 0
        for ch in cfg.chunks:
            for bidx in ch:
                srcs, ranks, st, en = per[(c, True)]
                cnt = en[bidx] - st[bidx]
                idx_lo[lo_off : lo_off + cnt] = srcs[st[bidx]:en[bidx]]
                pos = np.arange(cnt)
                rr = ranks[st[bidx]:en[bidx]]
                smat[pos % P, (bidx * TS + pos // P) * P + rr] = 1.0
                lo_off += TLs * P
            for bidx in ch:
                if THs == 0:
                    continue
                srcs, ranks, st, en = per[(c, False)]
                cnt = en[bidx] - st[bidx]
                idx_hi[hi_off : hi_off + cnt] = srcs[st[bidx]:en[bidx]] - cfg.LO
                pos = np.arange(cnt)
                rr = ranks[st[bidx]:en[bidx]]
                smat[pos % P, (bidx * TS + TLs + pos // P) * P + rr] = 1.0
                hi_off += THs * P

        ids = np.arange(c * NSH, (c + 1) * NSH)
        xin = np.zeros((NPAD, cfg.D), np.float32)
        xin[newlocal[ids]] = x[ids]
        degT = np.zeros((P, TPC), np.float32)
        degT[newlocal[ids] % P, newlocal[ids] // P] = deg[ids]

        in_maps.append(
            {
                "xin": xin,
                "wt": Wt,
                "bT": bT,
                "gamma": np.asarray(gamma, np.float32),
                "beta": np.asarray(beta, np.float32),
                "degT": degT,
                "maskv": maskv,
                "smat": smat,
                "idx_lo": _wrap16(idx_lo),
                "idx_hi": _wrap16(idx_hi),
            }
        )

    meta = dict(TLs=TLs, THs=THs, NT=NT, newlocal=newlocal)
    return in_maps, meta


def _build(cfg, TLs, THs):
    """Build the SPMD Bass program (identical for all cores)."""
    N, D, L, C = cfg.N, cfg.D, cfg.L, cfg.C
    TPC, NPAD, TROWS = cfg.TPC, cfg.NPAD, cfg.TROWS
    TS = TLs + THs
    NT = TPC * TS
    NTC_MAX = max(len(ch) for ch in cfg.chunks) * TS

    nc = bacc.Bacc("TRN2", target_bir_lowering=False, debug=False, num_devices=C)

    xin = nc.dram_tensor("xin", [NPAD, D], F32, kind="ExternalInput")
    wt = nc.dram_tensor("wt", [L, D, D], F32, kind="ExternalInput")
    bT = nc.dram_tensor("bT", [D, L], F32, kind="ExternalInput")
    gamma_d = nc.dram_tensor("gamma", [L, D], F32, kind="ExternalInput")
    beta_d = nc.dram_tensor("beta", [L, D], F32, kind="ExternalInput")
    degT = nc.dram_tensor("degT", [P, TPC], F32, kind="ExternalInput")
    maskv_d = nc.dram_tensor("maskv", [P, TPC], F32, kind="ExternalInput")
    smat_d = nc.dram_tensor("smat", [P, NT * P], BF16, kind="ExternalInput")
    idx_lo_d = nc.dram_tensor(
        "idx_lo", [P, TPC * TLs * P // 16], mybir.dt.int16, kind="ExternalInput"
    )
    nhi16 = max(TPC * THs * P, 16) // 16
    idx_hi_d = nc.dram_tensor(
        "idx_hi", [P, nhi16], mybir.dt.int16, kind="ExternalInput"
    )
    out_d = nc.dram_tensor("out", [NPAD, D], F32, kind="ExternalOutput")

    rg = [list(range(C))]

    with tile.TileContext(nc) as tc:
        with (
            tc.tile_pool(name="persist", bufs=1) as pp,
            tc.tile_pool(name="msgp", bufs=14) as msgp,
            tc.tile_pool(name="bigtmp", bufs=1) as btp,
            tc.tile_pool(name="sp", bufs=4) as sp,
            tc.tile_pool(name="work", bufs=4) as wp,
            tc.tile_pool(name="psblk", bufs=2, space="PSUM") as psblk,
            tc.tile_pool(name="psmisc", bufs=3, space="PSUM") as psmisc,
            tc.tile_pool(name="psbc", bufs=1, space="PSUM") as psbc,
            tc.tile_pool(name="psstat", bufs=2, space="PSUM") as psstat,
            tc.tile_pool(name="dram", bufs=1, space="DRAM") as dp,
        ):
            # ---- persistent loads ----
            x_sb = pp.tile([P, TPC, D], F32)
            nc.sync.dma_start(x_sb[:], xin[:].rearrange("(t p) f -> p t f", p=P))
            wt_sb = pp.tile([P, L, D], F32)
            for l in range(L):
                nc.sync.dma_start(wt_sb[:, l, :], wt[l, :, :])
            bT_sb = pp.tile([P, L], F32)
            nc.sync.dma_start(bT_sb[:], bT[:])
            gb_sb = pp.tile([1, 2 * L, D], F32)  # gamma/beta rows on partition 0
            for l in range(L):
                nc.sync.dma_start(gb_sb[:, l, :], gamma_d[l : l + 1, :])
                nc.sync.dma_start(gb_sb[:, L + l, :], beta_d[l : l + 1, :])
            deg_sb = pp.tile([P, TPC], F32)
            nc.sync.dma_start(deg_sb[:], degT[:])
            maskv_sb = pp.tile([P, TPC], F32)
            nc.sync.dma_start(maskv_sb[:], maskv_d[:])
            idx_lo_sb = pp.tile([P, TPC * TLs * P // 16], mybir.dt.int16)
            nc.sync.dma_start(idx_lo_sb[:], idx_lo_d[:])
            idx_hi_sb = pp.tile([P, nhi16], mybir.dt.int16)
            nc.sync.dma_start(idx_hi_sb[:], idx_hi_d[:])
            ident = pp.tile([P, P], F32)
            make_identity(nc, ident[:])
            ones1 = pp.tile([1, P], F32)
            nc.vector.memset(ones1[:], 1.0)

            # dinv = (deg > 0) / sqrt(max(deg, 1))
            dinv_sb = pp.tile([P, TPC], F32)
            t_a = wp.tile([P, TPC], F32, tag="dinv")
            nc.vector.tensor_scalar_max(t_a[:], deg_sb[:], 1.0)
            t_b = wp.tile([P, TPC], F32, tag="dinv")
            nc.vector.reciprocal(t_b[:], t_a[:])
            t_c = wp.tile([P, TPC], F32, tag="dinv")
            nc.scalar.sqrt(t_c[:], t_b[:])
            t_d = wp.tile([P, TPC], F32, tag="dinv")
            nc.vector.tensor_scalar(t_d[:], deg_sb[:], 0.0, None, ALU.is_gt)
            nc.vector.tensor_tensor(dinv_sb[:], t_c[:], t_d[:], ALU.mult)

            agg_sb = pp.tile([P, TPC, D], F32)
            hs_sb = pp.tile([P, TPC, D], BF16)

            # DRAM collective buffers
            shard_dr = dp.tile([NPAD, D], BF16)
            table_dr = dp.tile([TROWS, D], BF16)
            stats_in = dp.tile([1, 2 * D], F32)
            stats_out = dp.tile([1, 2 * D], F32)

            for l in range(L):
                # ---- hs = dinv * (x @ W^T + b), downcast bf16, row-major ----
                for t in range(TPC):
                    xT_ps = psmisc.tile([P, P], F32, tag="ps")
                    nc.tensor.transpose(xT_ps[:], x_sb[:, t, :], ident[:])
                    xT = wp.tile([P, P], F32, tag="xT")
                    nc.vector.tensor_copy(xT[:], xT_ps[:])
                    hT_ps = psmisc.tile([P, P], F32, tag="ps")
                    nc.tensor.matmul(
                        out=hT_ps[:], lhsT=wt_sb[:, l, :], rhs=xT[:],
                        start=True, stop=True,
                    )
                    hb = wp.tile([P, P], F32, tag="hb")
                    nc.scalar.activation(
                        hb[:], hT_ps[:], AF.Identity, bias=bT_sb[:, l : l + 1]
                    )
                    h_rm_ps = psmisc.tile([P, P], F32, tag="ps")
                    nc.tensor.transpose(h_rm_ps[:], hb[:], ident[:])
                    nc.scalar.activation(
                        hs_sb[:, t, :], h_rm_ps[:], AF.Identity,
                        scale=dinv_sb[:, t : t + 1],
                    )
                nc.sync.dma_start(
                    shard_dr[:].rearrange("(t p) f -> p t f", p=P), hs_sb[:]
                )
                nc.gpsimd.collective_compute(
                    "AllGather",
                    ALU.bypass,
                    ins=[shard_dr.opt()],
                    outs=[table_dr.opt()],
                    replica_groups=rg,
                )

                # ---- gather + one-hot matmul aggregation ----
                stA_ps = psstat.tile([1, P], F32, tag="st")
                stB_ps = psstat.tile([1, P], F32, tag="st")
                tile_col = 0
                lo_off = hi_off = 0
                for ch in cfg.chunks:
                    nb = len(ch)
                    ntc = nb * TS
                    # one msg tile per gather call (KG tiles each) for deep
                    # DMA pipelining via the pool; slot_of maps a chunk-local
                    # msg-tile column to its (pool tile, slot)
                    slot_of = {}

                    def _mt(mcol, _s=None):
                        mt, sl = slot_of[mcol]
                        return mt[:, sl, :]

                    nlo = nb * TLs * P
                    KGP = cfg.KG * P
                    for g0 in range(0, nlo, KGP):
                        g1 = min(g0 + KGP, nlo)
                        mt = msgp.tile([P, cfg.KG, D], BF16, tag="msg")
                        for i in range((g1 - g0) // P):
                            slot_of[g0 // P + i] = (mt, i)
                        nc.gpsimd.dma_gather(
                            mt[:, : (g1 - g0) // P, :],
                            table_dr[:],
                            idx_lo_sb[:, (lo_off + g0) // 16 : (lo_off + g1) // 16],
                            g1 - g0, g1 - g0, D,
                        )
                    lo_off += nlo
                    if THs > 0:
                        nhi = nb * THs * P
                        for g0 in range(0, nhi, KGP):
                            g1 = min(g0 + KGP, nhi)
                            mt = msgp.tile([P, cfg.KG, D], BF16, tag="msg")
                            for i in range((g1 - g0) // P):
                                slot_of[nb * TLs + g0 // P + i] = (mt, i)
                            nc.gpsimd.dma_gather(
                                mt[:, : (g1 - g0) // P, :],
                                table_dr[cfg.LO :, :],
                                idx_hi_sb[
                                    :, (hi_off + g0) // 16 : (hi_off + g1) // 16
                                ],
                                g1 - g0, g1 - g0, D,
                            )
                        hi_off += nhi
                    for j, bidx in enumerate(ch):
                        ps_b = psblk.tile([P, P], F32, tag="blk")
                        s_blk = sp.tile([P, TS, P], BF16, tag="s")
                        nc.sync.dma_start(
                            s_blk[:],
                            smat_d[:, bidx * TS * P : (bidx + 1) * TS * P],
                        )
                        mm, nmm = 0, TS
                        for t in range(TLs):
                            mcol = j * TLs + t
                            nc.tensor.matmul(
                                out=ps_b[:], lhsT=s_blk[:, t, :], rhs=_mt(mcol),
                                start=(mm == 0), stop=(mm == nmm - 1),
                            )
                            mm += 1
                        for t in range(THs):
                            mcol = nb * TLs + j * THs + t
                            nc.tensor.matmul(
                                out=ps_b[:], lhsT=s_blk[:, TLs + t, :], rhs=_mt(mcol),
                                start=(mm == 0), stop=(mm == nmm - 1),
                            )
                            mm += 1
                        nc.scalar.activation(
                            agg_sb[:, bidx, :], ps_b[:], AF.Identity,
                            scale=dinv_sb[:, bidx : bidx + 1],
                        )
                        nc.tensor.matmul(
                            out=stA_ps[:],
                            lhsT=maskv_sb[:, bidx : bidx + 1],
                            rhs=agg_sb[:, bidx, :],
                            start=(bidx == 0), stop=(bidx == TPC - 1),
                            skip_group_check=True,
                        )
                        aggsq = wp.tile([P, P], F32, tag="aggsq")
                        nc.scalar.square(aggsq[:], agg_sb[:, bidx, :])
                        nc.tensor.matmul(
                            out=stB_ps[:],
                            lhsT=maskv_sb[:, bidx : bidx + 1],
                            rhs=aggsq[:],
                            start=(bidx == 0), stop=(bidx == TPC - 1),
                            skip_group_check=True,
                        )
                    tile_col += ntc

                st_sb = wp.tile([1, 2, P], F32, tag="st")
                nc.vector.tensor_copy(st_sb[:, 0, :], stA_ps[:])
                nc.vector.tensor_copy(st_sb[:, 1, :], stB_ps[:])
                nc.sync.dma_start(stats_in[:], st_sb[:])
                nc.gpsimd.collective_compute(
                    "AllReduce",
                    ALU.add,
                    ins=[stats_in.opt()],
                    outs=[stats_out.opt()],
                    replica_groups=rg,
                )
                stg = wp.tile([1, 2, P], F32, tag="st")
                nc.sync.dma_start(stg[:], stats_out[:])

                # ---- scale/shift vectors on partition 0 ----
                vec = wp.tile([1, 8, P], F32, tag="vec")
                MU, MSQ, VAR, RSTD, SC, SH, T0, T1 = range(8)
                inv_n = 1.0 / float(N)
                nc.vector.tensor_scalar_mul(vec[:, MU, :], stg[:, 0, :], inv_n)
                nc.vector.tensor_scalar_mul(vec[:, MSQ, :], stg[:, 1, :], inv_n)
                nc.vector.tensor_tensor(
                    vec[:, T0, :], vec[:, MU, :], vec[:, MU, :], ALU.mult
                )
                nc.vector.tensor_tensor(
                    vec[:, VAR, :], vec[:, MSQ, :], vec[:, T0, :], ALU.subtract
                )
                nc.vector.tensor_scalar_add(vec[:, T1, :], vec[:, VAR, :], cfg.BN_EPS)
                nc.vector.reciprocal(vec[:, T0, :], vec[:, T1, :])
                nc.scalar.sqrt(vec[:, RSTD, :], vec[:, T0, :])
                nc.vector.tensor_tensor(
                    vec[:, SC, :], gb_sb[:, l, :], vec[:, RSTD, :], ALU.mult
                )
                nc.vector.tensor_tensor(
                    vec[:, T0, :], vec[:, MU, :], vec[:, SC, :], ALU.mult
                )
                nc.vector.tensor_tensor(
                    vec[:, SH, :], gb_sb[:, L + l, :], vec[:, T0, :], ALU.subtract
                )
                # broadcast scale|shift across partitions via ones-matmul
                bc_ps = psbc.tile([P, 2 * P], F32, tag="bc")
                nc.tensor.matmul(
                    out=bc_ps[:], lhsT=ones1[:], rhs=vec[:, SC : SH + 1, :],
                    start=True, stop=True,
                )
                screp = wp.tile([P, 2, P], F32, tag="screp")
                nc.vector.tensor_copy(screp[:], bc_ps[:])

                # ---- BN apply + relu + residual (whole shard) ----
                t1 = btp.tile([P, TPC, D], F32, tag="t1")
                nc.vector.tensor_tensor(
                    t1[:],
                    agg_sb[:],
                    screp[:, 0:1, :].to_broadcast([P, TPC, D]),
                    ALU.mult,
                )
                nc.vector.tensor_tensor(
                    t1[:],
                    t1[:],
                    screp[:, 1:2, :].to_broadcast([P, TPC, D]),
                    ALU.add,
                )
                nc.scalar.activation(t1[:], t1[:], AF.Relu)
                nc.vector.tensor_tensor(x_sb[:], x_sb[:], t1[:], ALU.add)

            nc.sync.dma_start(out_d[:].rearrange("(t p) f -> p t f", p=P), x_sb[:])

    nc.compile()
    return nc


_CACHE = {}


def _get_nc(cfg, TLs, THs):
    key = (cfg.N, cfg.E, cfg.L, cfg.C, cfg.BPC, cfg.KG, TLs, THs)
    if key not in _CACHE:
        _CACHE[key] = _build(cfg, TLs, THs)
    return _CACHE[key]


def run(cfg, inputs, trace=False):
    in_maps, meta = _preprocess(cfg, **inputs)
    nc = _get_nc(cfg, meta["TLs"], meta["THs"])
    res = run_bass_kernel_spmd(nc, in_maps, core_ids=list(range(cfg.C)), trace=trace)
    newlocal = meta["newlocal"]
    xfull = np.empty((cfg.N, cfg.D), np.float32)
    for c in range(cfg.C):
        ids = np.arange(c * cfg.NSH, (c + 1) * cfg.NSH)
        xfull[ids] = res.results[c]["out"][newlocal[ids]]
    return xfull, res


def kernel(x, edge_index, W, b, gamma, beta):
    cfg = Cfg(N=50000, E=800000, D=128, L=3, C=8, bpc=7, kg=8)
    out, _ = run(
        cfg, dict(x=x, edge_index=edge_index, W=W, b=b, gamma=gamma, beta=beta)
    )
    return out



# revision 9
# speedup vs baseline: 2.8200x; 2.8200x over previous
"""GCN message-passing kernel for 8 Trainium2 NeuronCores.

Strategy (graph/data parallel, per the sharding hint):
  - Destination nodes are sharded across the 8 cores in contiguous ranges.
  - Within each core, its destinations are dealt (by in-degree, snake order)
    into 128-wide blocks so per-block edge counts are balanced across
    blocks AND cores (the SPMD program has compile-time-fixed loop bounds).
  - Per layer: each core computes hs = dinv * (x W^T + b) for its own node
    shard (PE transpose + matmul), downcasts to bf16, and the shards are
    AllGathered into a full [C*NPAD, 128] bf16 table in DRAM.
  - Messages are fetched with batched indirect DMA gathers (one SWDGE
    instruction per ~hundred 128-edge tiles) and scatter-added per
    destination block with a one-hot matmul:
        agg_block[d, f] += S_tile[e, d]^T @ msg_tile[e, f]
    accumulated in PSUM. S_tile is built on the DVE with a single
    is_equal tensor_scalar of an iota row against per-edge dest ranks.
  - BN statistics (sum, sum of squares) are computed with mask-vector
    matmuls over the aggregated blocks and AllReduced across cores; the
    apply (scale/shift + relu + residual) runs on full-shard DVE/ACT ops.

kernel(**inputs) takes the FULL inputs and returns the FULL output.
"""

import numpy as np
import ml_dtypes

import concourse.bacc as bacc
import concourse.bass as bass
import concourse.mybir as mybir
import concourse.tile as tile
from concourse.bass_utils import run_bass_kernel_spmd
from concourse.masks import make_identity

P = 128
F32 = mybir.dt.float32
BF16 = mybir.dt.bfloat16
AF = mybir.ActivationFunctionType
ALU = mybir.AluOpType


class Cfg:
    def __init__(self, N, E, D, L, C, bpc, kg=4, nq=4, bn_eps=1e-5):
        assert D == 128
        self.N, self.E, self.D, self.L, self.C = N, E, D, L, C
        self.NSH = N // C                      # real nodes per core
        assert self.NSH * C == N
        self.TPC = (self.NSH + P - 1) // P     # node tiles (blocks) per core
        self.NPAD = self.TPC * P               # padded nodes per core
        assert self.NSH < self.NPAD, "need at least one guaranteed-zero pad row"
        self.TROWS = C * self.NPAD             # gather table rows
        self.BPC = bpc                         # blocks per gather chunk
        self.chunks = [
            list(range(i, min(i + bpc, self.TPC))) for i in range(0, self.TPC, bpc)
        ]
        self.BN_EPS = bn_eps
        self.KG = kg  # max idxs per dma_gather call (in 128-edge tiles)
        self.NQ = nq
        self.ZROW = self.NSH  # core 0's first pad row: always written as zero
        self.LO = 32768
        if self.TROWS > self.LO:
            c_hi = -((self.LO - self.NSH) // -self.NPAD)
            zhi = c_hi * self.NPAD + self.NSH
            assert self.LO <= zhi < self.TROWS
            self.ZHI = zhi - self.LO
        else:
            self.ZHI = 0


def _preprocess(cfg, x, edge_index, W, b, gamma, beta):
    """All index/layout work on the host. Returns per-core in_maps and the
    (identical across cores) compile-time tile structure."""
    N, C, NSH, NPAD, TPC = cfg.N, cfg.C, cfg.NSH, cfg.NPAD, cfg.TPC
    row = np.asarray(edge_index[0], dtype=np.int64)
    col = np.asarray(edge_index[1], dtype=np.int64)
    x = np.asarray(x, dtype=np.float32)
    deg = np.bincount(row, minlength=N).astype(np.float32)  # out-degree
    deg_in = np.bincount(col, minlength=N)

    # Per-core local permutation: snake-deal destinations (sorted by
    # in-degree desc) into TPC blocks -> balanced per-block edge counts.
    newlocal = np.empty(N, np.int64)
    nblk0 = None
    for c in range(C):
        ids = np.arange(c * NSH, (c + 1) * NSH)
        order = ids[np.argsort(-deg_in[ids], kind="stable")]
        i = np.arange(NSH)
        r, j = i // TPC, i % TPC
        blk = np.where(r % 2 == 1, TPC - 1 - j, j)
        rank = np.zeros(NSH, np.int64)
        cnt = np.zeros(TPC, np.int64)
        for k in range(NSH):
            rank[k] = cnt[blk[k]]
            cnt[blk[k]] += 1
        newlocal[order] = blk * P + rank
        if nblk0 is None:
            nblk0 = cnt.copy()
        else:
            assert (cnt == nblk0).all()
    assert nblk0.max() <= P

    maskv = (np.arange(P)[:, None] < nblk0[None, :]).astype(np.float32)
    table_row = (np.arange(N) // NSH) * NPAD + newlocal  # node -> table row

    e_core = col // NSH
    e_blk = newlocal[col] // P
    e_rank = newlocal[col] % P
    e_src = table_row[row]

    # common tile structure: TLs/THs tiles per block, max over cores/blocks
    split_hi = cfg.TROWS > cfg.LO
    per = {}
    TLs, THs = 1, (1 if split_hi else 0)
    for c in range(C):
        selc = e_core == c
        for lo in (True, False):
            if not lo and not split_hi:
                continue
            sel = selc & ((e_src < cfg.LO) == lo)
            srcs, blks, ranks = e_src[sel], e_blk[sel], e_rank[sel]
            o = np.argsort(blks, kind="stable")
            srcs, blks, ranks = srcs[o], blks[o], ranks[o]
            starts = np.searchsorted(blks, np.arange(TPC))
            ends = np.searchsorted(blks, np.arange(TPC) + 1)
            per[(c, lo)] = (srcs, ranks, starts, ends)
            m = int((-((ends - starts) // -P)).max())
            if lo:
                TLs = max(TLs, m)
            else:
                THs = max(THs, m)
    if not split_hi:
        per = {(c, True): per[(c, True)] for c in range(C)}
    TS = TLs + THs
    NT = TPC * TS
    in_maps = []
    Wt = np.ascontiguousarray(np.transpose(np.asarray(W, np.float32), (0, 2, 1)))
    bT = np.ascontiguousarray(np.asarray(b, np.float32).T)

    def _wrap16(idx):
        w = idx.reshape(-1, 16).T.astype(np.int16)
        return np.ascontiguousarray(np.tile(w, (8, 1)))

    for c in range(C):
        idx_lo = np.full(TPC * TLs * P, cfg.ZROW, np.int64)
        idx_hi = np.full(max(TPC * THs * P, 16), cfg.ZHI, np.int64)
        # one-hot S matrices, block-contiguous: smat[e, (b*TS + t)*P + d]
        smat = np.zeros((P, NT * P), ml_dtypes.bfloat16)
        lo_off = hi_off = 0
        for ch in cfg.chunks:
            for bidx in ch:
                srcs, ranks, st, en = per[(c, True)]
                cnt = en[bidx] - st[bidx]
                idx_lo[lo_off : lo_off + cnt] = srcs[st[bidx]:en[bidx]]
                pos = np.arange(cnt)
                rr = ranks[st[bidx]:en[bidx]]
                smat[pos % P, (bidx * TS + pos // P) * P + rr] = 1.0
                lo_off += TLs * P
            for bidx in ch:
                if THs == 0:
                    continue
                srcs, ranks, st, en = per[(c, False)]
                cnt = en[bidx] - st[bidx]
                idx_hi[hi_off : hi_off + cnt] = srcs[st[bidx]:en[bidx]] - cfg.LO
                pos = np.arange(cnt)
                rr = ranks[st[bidx]:en[bidx]]
                smat[pos % P, (bidx * TS + TLs + pos // P) * P + rr] = 1.0
                hi_off += THs * P

        ids = np.arange(c * NSH, (c + 1) * NSH)
        xin = np.zeros((NPAD, cfg.D), np.float32)
        xin[newlocal[ids]] = x[ids]
        degT = np.zeros((P, TPC), np.float32)
        degT[newlocal[ids] % P, newlocal[ids] // P] = deg[ids]

        in_maps.append(
            {
                "xin": xin,
                "wt": Wt,
                "bT": bT,
                "gamma": np.asarray(gamma, np.float32),
                "beta": np.asarray(beta, np.float32),
                "degT": degT,
                "maskv": maskv,
                "smat": smat,
                "idx_lo": _wrap16(idx_lo),
                "idx_hi": _wrap16(idx_hi),
            }
        )

    meta = dict(TLs=TLs, THs=THs, NT=NT, newlocal=newlocal)
    return in_maps, meta


def _build(cfg, TLs, THs):
    """Build the SPMD Bass program (identical for all cores)."""
    N, D, L, C = cfg.N, cfg.D, cfg.L, cfg.C
    TPC, NPAD, TROWS = cfg.TPC, cfg.NPAD, cfg.TROWS
    TS = TLs + THs
    NT = TPC * TS
    NTC_MAX = max(len(ch) for ch in cfg.chunks) * TS

    nc = bacc.Bacc("TRN2", target_bir_lowering=False, debug=False, num_devices=C,
                   num_swdge_queues=cfg.NQ)

    xin = nc.dram_tensor("xin", [NPAD, D], F32, kind="ExternalInput")
    wt = nc.dram_tensor("wt", [L, D, D], F32, kind="ExternalInput")
    bT = nc.dram_tensor("bT", [D, L], F32, kind="ExternalInput")
    gamma_d = nc.dram_tensor("gamma", [L, D], F32, kind="ExternalInput")
    beta_d = nc.dram_tensor("beta", [L, D], F32, kind="ExternalInput")
    degT = nc.dram_tensor("degT", [P, TPC], F32, kind="ExternalInput")
    maskv_d = nc.dram_tensor("maskv", [P, TPC], F32, kind="ExternalInput")
    smat_d = nc.dram_tensor("smat", [P, NT * P], BF16, kind="ExternalInput")
    idx_lo_d = nc.dram_tensor(
        "idx_lo", [P, TPC * TLs * P // 16], mybir.dt.int16, kind="ExternalInput"
    )
    nhi16 = max(TPC * THs * P, 16) // 16
    idx_hi_d = nc.dram_tensor(
        "idx_hi", [P, nhi16], mybir.dt.int16, kind="ExternalInput"
    )
    out_d = nc.dram_tensor("out", [NPAD, D], F32, kind="ExternalOutput")

    rg = [list(range(C))]

    with tile.TileContext(nc) as tc:
        with (
            tc.tile_pool(name="persist", bufs=1) as pp,
            tc.tile_pool(name="msgp", bufs=14) as msgp,
            tc.tile_pool(name="bigtmp", bufs=1) as btp,
            tc.tile_pool(name="sp", bufs=4) as sp,
            tc.tile_pool(name="work", bufs=4) as wp,
            tc.tile_pool(name="psblk", bufs=2, space="PSUM") as psblk,
            tc.tile_pool(name="psmisc", bufs=3, space="PSUM") as psmisc,
            tc.tile_pool(name="psbc", bufs=1, space="PSUM") as psbc,
            tc.tile_pool(name="psstat", bufs=2, space="PSUM") as psstat,
            tc.tile_pool(name="dram", bufs=1, space="DRAM") as dp,
        ):
            # ---- persistent loads ----
            x_sb = pp.tile([P, TPC, D], F32)
            nc.sync.dma_start(x_sb[:], xin[:].rearrange("(t p) f -> p t f", p=P))
            wt_sb = pp.tile([P, L, D], F32)
            for l in range(L):
                nc.sync.dma_start(wt_sb[:, l, :], wt[l, :, :])
            bT_sb = pp.tile([P, L], F32)
            nc.sync.dma_start(bT_sb[:], bT[:])
            gb_sb = pp.tile([1, 2 * L, D], F32)  # gamma/beta rows on partition 0
            for l in range(L):
                nc.sync.dma_start(gb_sb[:, l, :], gamma_d[l : l + 1, :])
                nc.sync.dma_start(gb_sb[:, L + l, :], beta_d[l : l + 1, :])
            deg_sb = pp.tile([P, TPC], F32)
            nc.sync.dma_start(deg_sb[:], degT[:])
            maskv_sb = pp.tile([P, TPC], F32)
            nc.sync.dma_start(maskv_sb[:], maskv_d[:])
            idx_lo_sb = pp.tile([P, TPC * TLs * P // 16], mybir.dt.int16)
            nc.sync.dma_start(idx_lo_sb[:], idx_lo_d[:])
            idx_hi_sb = pp.tile([P, nhi16], mybir.dt.int16)
            nc.sync.dma_start(idx_hi_sb[:], idx_hi_d[:])
            ident = pp.tile([P, P], F32)
            make_identity(nc, ident[:])
            ones1 = pp.tile([1, P], F32)
            nc.vector.memset(ones1[:], 1.0)

            # dinv = (deg > 0) / sqrt(max(deg, 1))
            dinv_sb = pp.tile([P, TPC], F32)
            t_a = wp.tile([P, TPC], F32, tag="dinv")
            nc.vector.tensor_scalar_max(t_a[:], deg_sb[:], 1.0)
            t_b = wp.tile([P, TPC], F32, tag="dinv")
            nc.vector.reciprocal(t_b[:], t_a[:])
            t_c = wp.tile([P, TPC], F32, tag="dinv")
            nc.scalar.sqrt(t_c[:], t_b[:])
            t_d = wp.tile([P, TPC], F32, tag="dinv")
            nc.vector.tensor_scalar(t_d[:], deg_sb[:], 0.0, None, ALU.is_gt)
            nc.vector.tensor_tensor(dinv_sb[:], t_c[:], t_d[:], ALU.mult)

            agg_sb = pp.tile([P, TPC, D], F32)
            hs_sb = pp.tile([P, TPC, D], BF16)

            # DRAM collective buffers
            shard_dr = dp.tile([NPAD, D], BF16)
            table_dr = dp.tile([TROWS, D], BF16)
            stats_in = dp.tile([1, 2 * D], F32)
            stats_out = dp.tile([1, 2 * D], F32)

            for l in range(L):
                # ---- hs = dinv * (x @ W^T + b), downcast bf16, row-major ----
                for t in range(TPC):
                    xT_ps = psmisc.tile([P, P], F32, tag="ps")
                    nc.tensor.transpose(xT_ps[:], x_sb[:, t, :], ident[:])
                    xT = wp.tile([P, P], F32, tag="xT")
                    nc.vector.tensor_copy(xT[:], xT_ps[:])
                    hT_ps = psmisc.tile([P, P], F32, tag="ps")
                    nc.tensor.matmul(
                        out=hT_ps[:], lhsT=wt_sb[:, l, :], rhs=xT[:],
                        start=True, stop=True,
                    )
                    hb = wp.tile([P, P], F32, tag="hb")
                    nc.scalar.activation(
                        hb[:], hT_ps[:], AF.Identity, bias=bT_sb[:, l : l + 1]
                    )
                    h_rm_ps = psmisc.tile([P, P], F32, tag="ps")
                    nc.tensor.transpose(h_rm_ps[:], hb[:], ident[:])
                    nc.scalar.activation(
                        hs_sb[:, t, :], h_rm_ps[:], AF.Identity,
                        scale=dinv_sb[:, t : t + 1],
                    )
                nc.sync.dma_start(
                    shard_dr[:].rearrange("(t p) f -> p t f", p=P), hs_sb[:]
                )
                nc.gpsimd.collective_compute(
                    "AllGather",
                    ALU.bypass,
                    ins=[shard_dr.opt()],
                    outs=[table_dr.opt()],
                    replica_groups=rg,
                )

                # ---- gather + one-hot matmul aggregation ----
                stA_ps = psstat.tile([1, P], F32, tag="st")
                stB_ps = psstat.tile([1, P], F32, tag="st")
                tile_col = 0
                lo_off = hi_off = 0
                ncall = 0
                for ch in cfg.chunks:
                    nb = len(ch)
                    ntc = nb * TS
                    # one msg tile per gather call (KG tiles each) for deep
                    # DMA pipelining via the pool; slot_of maps a chunk-local
                    # msg-tile column to its (pool tile, slot)
                    slot_of = {}

                    def _mt(mcol, _s=None):
                        mt, sl = slot_of[mcol]
                        return mt[:, sl, :]

                    nlo = nb * TLs * P
                    KGP = cfg.KG * P
                    for g0 in range(0, nlo, KGP):
                        g1 = min(g0 + KGP, nlo)
                        mt = msgp.tile([P, cfg.KG, D], BF16, tag="msg")
                        for i in range((g1 - g0) // P):
                            slot_of[g0 // P + i] = (mt, i)
                        nc.gpsimd.dma_gather(
                            mt[:, : (g1 - g0) // P, :],
                            table_dr[:],
                            idx_lo_sb[:, (lo_off + g0) // 16 : (lo_off + g1) // 16],
                            g1 - g0, g1 - g0, D,
                            queue_num=ncall % cfg.NQ,
                        )
                        ncall += 1
                    lo_off += nlo
                    if THs > 0:
                        nhi = nb * THs * P
                        for g0 in range(0, nhi, KGP):
                            g1 = min(g0 + KGP, nhi)
                            mt = msgp.tile([P, cfg.KG, D], BF16, tag="msg")
                            for i in range((g1 - g0) // P):
                                slot_of[nb * TLs + g0 // P + i] = (mt, i)
                            nc.gpsimd.dma_gather(
                                mt[:, : (g1 - g0) // P, :],
                                table_dr[cfg.LO :, :],
                                idx_hi_sb[
                                    :, (hi_off + g0) // 16 : (hi_off + g1) // 16
                                ],
                                g1 - g0, g1 - g0, D,
                                queue_num=ncall % cfg.NQ,
                            )
                            ncall += 1
                        hi_off += nhi
                    for j, bidx in enumerate(ch):
                        ps_b = psblk.tile([P, P], F32, tag="blk")
                        s_blk = sp.tile([P, TS, P], BF16, tag="s")
                        nc.sync.dma_start(
                            s_blk[:],
                            smat_d[:, bidx * TS * P : (bidx + 1) * TS * P],
                        )
                        mm, nmm = 0, TS
                        for t in range(TLs):
                            mcol = j * TLs + t
                            nc.tensor.matmul(
                                out=ps_b[:], lhsT=s_blk[:, t, :], rhs=_mt(mcol),
                                start=(mm == 0), stop=(mm == nmm - 1),
                            )
                            mm += 1
                        for t in range(THs):
                            mcol = nb * TLs + j * THs + t
                            nc.tensor.matmul(
                                out=ps_b[:], lhsT=s_blk[:, TLs + t, :], rhs=_mt(mcol),
                                start=(mm == 0), stop=(mm == nmm - 1),
                            )
                            mm += 1
                        nc.scalar.activation(
                            agg_sb[:, bidx, :], ps_b[:], AF.Identity,
                            scale=dinv_sb[:, bidx : bidx + 1],
                        )
                        nc.tensor.matmul(
                            out=stA_ps[:],
                            lhsT=maskv_sb[:, bidx : bidx + 1],
                            rhs=agg_sb[:, bidx, :],
                            start=(bidx == 0), stop=(bidx == TPC - 1),
                            skip_group_check=True,
                        )
                        aggsq = wp.tile([P, P], F32, tag="aggsq")
                        nc.scalar.square(aggsq[:], agg_sb[:, bidx, :])
                        nc.tensor.matmul(
                            out=stB_ps[:],
                            lhsT=maskv_sb[:, bidx : bidx + 1],
                            rhs=aggsq[:],
                            start=(bidx == 0), stop=(bidx == TPC - 1),
                            skip_group_check=True,
                        )
                    tile_col += ntc

                st_sb = wp.tile([1, 2, P], F32, tag="st")
                nc.vector.tensor_copy(st_sb[:, 0, :], stA_ps[:])
                nc.vector.tensor_copy(st_sb[:, 1, :], stB_ps[:])
                nc.sync.dma_start(stats_in[:], st_sb[:])
                nc.gpsimd.collective_compute(
                    "AllReduce",
                    ALU.add,
                    ins=[stats_in.opt()],
                    outs=[stats_out.opt()],
                    replica_groups=rg,
                )
                stg = wp.tile([1, 2, P], F32, tag="st")
                nc.sync.dma_start(stg[:], stats_out[:])

                # ---- scale/shift vectors on partition 0 ----
                vec = wp.tile([1, 8, P], F32, tag="vec")
                MU, MSQ, VAR, RSTD, SC, SH, T0, T1 = range(8)
                inv_n = 1.0 / float(N)
                nc.vector.tensor_scalar_mul(vec[:, MU, :], stg[:, 0, :], inv_n)
                nc.vector.tensor_scalar_mul(vec[:, MSQ, :], stg[:, 1, :], inv_n)
                nc.vector.tensor_tensor(
                    vec[:, T0, :], vec[:, MU, :], vec[:, MU, :], ALU.mult
                )
                nc.vector.tensor_tensor(
                    vec[:, VAR, :], vec[:, MSQ, :], vec[:, T0, :], ALU.subtract
                )
                nc.vector.tensor_scalar_add(vec[:, T1, :], vec[:, VAR, :], cfg.BN_EPS)
                nc.vector.reciprocal(vec[:, T0, :], vec[:, T1, :])
                nc.scalar.sqrt(vec[:, RSTD, :], vec[:, T0, :])
                nc.vector.tensor_tensor(
                    vec[:, SC, :], gb_sb[:, l, :], vec[:, RSTD, :], ALU.mult
                )
                nc.vector.tensor_tensor(
                    vec[:, T0, :], vec[:, MU, :], vec[:, SC, :], ALU.mult
                )
                nc.vector.tensor_tensor(
                    vec[:, SH, :], gb_sb[:, L + l, :], vec[:, T0, :], ALU.subtract
                )
                # broadcast scale|shift across partitions via ones-matmul
                bc_ps = psbc.tile([P, 2 * P], F32, tag="bc")
                nc.tensor.matmul(
                    out=bc_ps[:], lhsT=ones1[:], rhs=vec[:, SC : SH + 1, :],
                    start=True, stop=True,
                )
                screp = wp.tile([P, 2, P], F32, tag="screp")
                nc.vector.tensor_copy(screp[:], bc_ps[:])

                # ---- BN apply + relu + residual (whole shard) ----
                t1 = btp.tile([P, TPC, D], F32, tag="t1")
                nc.vector.tensor_tensor(
                    t1[:],
                    agg_sb[:],
                    screp[:, 0:1, :].to_broadcast([P, TPC, D]),
                    ALU.mult,
                )
                nc.vector.tensor_tensor(
                    t1[:],
                    t1[:],
                    screp[:, 1:2, :].to_broadcast([P, TPC, D]),
                    ALU.add,
                )
                nc.scalar.activation(t1[:], t1[:], AF.Relu)
                nc.vector.tensor_tensor(x_sb[:], x_sb[:], t1[:], ALU.add)

            nc.sync.dma_start(out_d[:].rearrange("(t p) f -> p t f", p=P), x_sb[:])

    nc.compile()
    return nc


_CACHE = {}


def _get_nc(cfg, TLs, THs):
    key = (cfg.N, cfg.E, cfg.L, cfg.C, cfg.BPC, cfg.KG, cfg.NQ, TLs, THs)
    if key not in _CACHE:
        _CACHE[key] = _build(cfg, TLs, THs)
    return _CACHE[key]


def run(cfg, inputs, trace=False):
    in_maps, meta = _preprocess(cfg, **inputs)
    nc = _get_nc(cfg, meta["TLs"], meta["THs"])
    res = run_bass_kernel_spmd(nc, in_maps, core_ids=list(range(cfg.C)), trace=trace)
    newlocal = meta["newlocal"]
    xfull = np.empty((cfg.N, cfg.D), np.float32)
    for c in range(cfg.C):
        ids = np.arange(c * cfg.NSH, (c + 1) * cfg.NSH)
        xfull[ids] = res.results[c]["out"][newlocal[ids]]
    return xfull, res


def kernel(x, edge_index, W, b, gamma, beta):
    cfg = Cfg(N=50000, E=800000, D=128, L=3, C=8, bpc=7, kg=8)
    out, _ = run(
        cfg, dict(x=x, edge_index=edge_index, W=W, b=b, gamma=gamma, beta=beta)
    )
    return out

